# revision 18
# baseline (speedup 1.0000x reference)
"""CBAM3D Trainium2 kernel (8 NeuronCores, SPMD).

Reference computation (per batch sample b):
  avg_pool[c] = mean_{d,h,w} x ; max_pool[c] = max_{d,h,w} x
  ca = sigmoid(relu(avg@w1+b1)@w2+b2) + sigmoid(relu(max@w1+b1)@w2+b2)
  refined = x * ca[c]
  P = [mean_c refined, max_c refined]            # [D,H,W,2]
  sa = sigmoid(conv3d_same(P, conv_w))           # 7x7x7x2 -> 1
  out = refined * sa

Sharding: core i handles sample b=i//2, D-half half=i%2 (32 planes, NO host
halo padding). Cross-core traffic: a pair-wise AllGather of channel sum/max
stats (512B) and a pair-wise AllGather of the 3-slot pooled-map halo
(~108KB) — the full-resolution x halo is never re-read from HBM.

Per-core pipeline (engine balance is the whole game; DVE is the wall):
  pass1: stream x f32 (plane-pair tiles), cast to a bf16 SBUF cache on the
         Scalar engine, accumulate channel sum (PE matmul vs ones) and
         channel max (running elementwise max on DVE, 2x)
  AllGather stats over {2i,2i+1}; transpose-free tiny MLP on device -> ca
  phase2a per pair: refined = cache*ca in-place (DVE 2x); channel SUM and
         MAX trees (DVE 2x + one 1x reduce each); pooled map
         [(ci,h'), slot, w+pad] via permutation matmuls (PE) + psum->pooled
         copies (ACT). Edge pairs go first; then the pooled halo exchange
         (AllGather + parity-conditional DMAs into the halo slots).
  conv:  49 taps x 4 blocks (8 planes) of accumulating matmuls with
         host-prebuilt band matrices (kh,ci folded into K=128) -> sigmoid
         -> sa stored C-pair-duplicated so the apply runs at DVE 2x
         (a stride-0 broadcast would force 1x — measured 2.2x slower)
  apply: cache *= sa in-place (DVE 2x), one DMA per plane-pair to HBM bf16
"""

from dataclasses import dataclass

import numpy as np
import ml_dtypes

import concourse.bass as bass
import concourse.tile as tile
import concourse.mybir as mybir
from concourse import bacc, bass_isa

F32 = mybir.dt.float32
BF16 = mybir.dt.bfloat16
AX = mybir.AxisListType
OP = mybir.AluOpType
ACT = mybir.ActivationFunctionType


@dataclass(frozen=True)
class Cfg:
    H: int = 64
    W: int = 64
    C: int = 64
    D_LOC: int = 32          # own planes per core
    HID: int = 4             # C // reduction_ratio
    KS: int = 7
    N_CORES: int = 8
    use_collectives: bool = True
    stop_after: str = "full"   # pass1 | mlp | full

    @property
    def HALO(self):
        return self.KS // 2

    @property
    def S(self):
        return self.D_LOC + 2 * self.HALO   # slots in the pooled map

    @property
    def P(self):
        return 2 * self.H                    # partition dim of pair tiles

    @property
    def WP(self):
        return self.W + 2 * self.HALO        # padded pooled-map width

    @property
    def D_TOT(self):
        return 2 * self.D_LOC                # full-sample depth (2 shards)


FULL = Cfg()


def _bc(ap, shape, axis):
    """broadcast ap (by unsqueezing `axis`) to `shape`"""
    return ap.unsqueeze(axis).broadcast_to(shape)


def build_cbam(nc, cfg: Cfg):
    H, W, C = cfg.H, cfg.W, cfg.C
    P, S, WP, HALO = cfg.P, cfg.S, cfg.WP, cfg.HALO
    D_LOC, HID, KS = cfg.D_LOC, cfg.HID, cfg.KS
    PAIRS = D_LOC // 2
    BLK = 8                                  # d-planes per conv block
    NB = D_LOC // BLK
    PPB = BLK // 2                           # plane-pairs per conv block
    W2 = W // 2
    NT = KS * KS

    xs = nc.dram_tensor("xs", [D_LOC, H, W, C], F32, kind="ExternalInput").ap()
    w1 = nc.dram_tensor("w1", [C, HID], F32, kind="ExternalInput").ap()
    b1t = nc.dram_tensor("b1t", [HID, 1], F32, kind="ExternalInput").ap()
    w2 = nc.dram_tensor("w2", [HID, C], F32, kind="ExternalInput").ap()
    b2 = nc.dram_tensor("b2", [1, C], F32, kind="ExternalInput").ap()
    sband = nc.dram_tensor("sband", [P, NT, H], BF16, kind="ExternalInput").ap()
    out_t = nc.dram_tensor("out", [D_LOC, H, W, C], BF16, kind="ExternalOutput").ap()

    groups = [[i, i + 1] for i in range(0, cfg.N_CORES, 2)]

    with tile.TileContext(nc) as tc:
        with (
            tc.tile_pool(name="consts", bufs=1) as consts,
            tc.tile_pool(name="cache", bufs=1) as cachep,
            tc.tile_pool(name="tree", bufs=1) as treep,
            tc.tile_pool(name="work", bufs=2) as workp,
            tc.tile_pool(name="dram", bufs=1, space="DRAM") as dram,
            tc.tile_pool(name="ps_stats", bufs=1, space="PSUM") as ps_stats,
            tc.tile_pool(name="ps_perm", bufs=2, space="PSUM") as ps_perm,
            tc.tile_pool(name="ps_psp", bufs=2, space="PSUM") as ps_psp,
            tc.tile_pool(name="ps_cv", bufs=2, space="PSUM") as ps_cv,
            tc.tile_pool(name="ps_sm", bufs=1, space="PSUM") as ps_sm,
        ):
            # ---------------- constants ----------------
            ones = consts.tile([P, 1], BF16, tag="ones")
            nc.vector.memset(ones, 1.0)

            # bf16 permutation matrices; pooled partition layout is (ci*H+h').
            def diag(t, col_lo, col_hi, base):
                nc.gpsimd.affine_select(
                    out=t[:, col_lo:col_hi], in_=t[:, col_lo:col_hi],
                    compare_op=OP.not_equal, fill=1.0, base=base,
                    pattern=[[-1, col_hi - col_lo]], channel_multiplier=1)

            qa_e = consts.tile([P, P], BF16, tag="qa_e")
            qb_e = consts.tile([P, P], BF16, tag="qb_e")
            qa_o = consts.tile([P, P], BF16, tag="qa_o")
            qb_o = consts.tile([P, P], BF16, tag="qb_o")
            for t in (qa_e, qb_e, qa_o, qb_o):
                nc.gpsimd.memset(t, 0.0)
            diag(qa_e, 0, H, 0)
            diag(qb_e, H, P, 0)
            diag(qa_o, 0, H, -H)
            diag(qb_o, H, P, -H)

            sband_sb = consts.tile([P, NT, H], BF16, tag="sband")
            nc.sync.dma_start(
                out=sband_sb[:].rearrange("p t h -> p (t h)"),
                in_=sband.rearrange("p t h -> p (t h)"))
            w1_sb = consts.tile([C, HID], F32, tag="w1")
            nc.sync.dma_start(out=w1_sb, in_=w1)
            w2_sb = consts.tile([HID, C], F32, tag="w2")
            nc.sync.dma_start(out=w2_sb, in_=w2)
            b1t_sb = consts.tile([HID, 1], F32, tag="b1t")
            nc.sync.dma_start(out=b1t_sb, in_=b1t)

            def dma_bcast(dst, src_ap, parts):
                a = bass.AP(tensor=src_ap.tensor, offset=src_ap.offset,
                            ap=[[0, parts]] + [list(p) for p in src_ap.ap[1:]])
                nc.sync.dma_start(out=dst, in_=a)

            b2b = consts.tile([2, C], F32, tag="b2")
            dma_bcast(b2b, b2, 2)

            # pre-warm the ACT table set (Relu/Sigmoid) so the first real
            # activation in the latency-critical MLP doesn't pay the load
            warm = consts.tile([1, 1], F32, tag="warm")
            nc.scalar.activation(out=warm, in_=b2b[0:1, 0:1], func=ACT.Relu)
            nc.scalar.activation(out=warm, in_=warm, func=ACT.Sigmoid)
            ones12 = consts.tile([1, 2], F32, tag="ones12")
            nc.vector.memset(ones12, 1.0)

            if cfg.use_collectives:
                wu_s = dram.tile([1, 1], F32, tag="wu_s")
                wu_r = dram.tile([2, 1], F32, tag="wu_r")
                nc.gpsimd.dma_start(out=wu_s, in_=b2b[0:1, 0:1])
                nc.gpsimd.collective_compute(
                    "AllGather", OP.bypass, replica_groups=groups,
                    ins=[wu_s.opt()], outs=[wu_r.opt()])

            # persistent state. pair j covers planes (2j, 2j+1) -> pooled
            # slots (HALO+2j, HALO+2j+1). Halo slots 0:3 / 35:38 come from
            # the neighbor core (or stay zero at sample boundaries).
            cache = [cachep.tile([P, W, C], BF16, tag=f"cache{j}",
                                 name=f"cache{j}") for j in range(PAIRS)]
            acc_max = cachep.tile([P, W2, C], BF16, tag="acc_max")
            nc.vector.memset(acc_max, -3.0e38)
            pooled = cachep.tile([P, S, WP], BF16, tag="pooled")
            nc.gpsimd.memset(pooled, 0.0)
            sa_sb = [cachep.tile([H, BLK, W], BF16, tag=f"sa{b}", name=f"sa{b}")
                     for b in range(NB)]
            # sa duplicated along a trailing len-2 axis: the apply
            # tensor_tensor then reads packed bf16 pairs (2x DVE mode).
            sa_dup = [cachep.tile([P, PPB, W, 2], BF16, tag=f"sad{b}",
                                  name=f"sad{b}") for b in range(NB)]

            # ---------------- pass 1: casting loads + stats ----------------
            # gpsimd (SWDGE) DMAs cast f32->bf16 inline: the HBM stream
            # lands directly in the bf16 cache — no stage tiles, no ACT
            # cast pass.
            psum_stats = ps_stats.tile([1, 8, C], F32, tag="stats")
            n_wg = W // 8
            mm_i = 0
            n_mm = PAIRS * n_wg
            for j in range(PAIRS):
                for wh in range(2):
                    nc.gpsimd.dma_start(
                        out=cache[j][:, wh * W2:(wh + 1) * W2, :]
                        .rearrange("p w c -> p (w c)"),
                        in_=xs[2 * j:2 * j + 2, :, wh * W2:(wh + 1) * W2, :]
                        .rearrange("d h w c -> (d h) (w c)"))
                    # channel max: running elementwise max over half tiles
                    nc.vector.tensor_tensor(
                        out=acc_max[:].rearrange("p w c -> p (w c)"),
                        in0=acc_max[:].rearrange("p w c -> p (w c)"),
                        in1=cache[j][:, wh * W2:(wh + 1) * W2, :]
                        .rearrange("p w c -> p (w c)"),
                        op=OP.max)
                for g in range(n_wg):
                    nc.tensor.matmul(
                        out=psum_stats,
                        lhsT=ones[:, :],
                        rhs=cache[j][:, g * 8:(g + 1) * 8, :],
                        start=(mm_i == 0), stop=(mm_i == n_mm - 1))
                    mm_i += 1

            # finalize stats (mean scale applied here, off the critical path)
            sumc = workp.tile([1, C], F32, tag="sumc", bufs=1)
            nc.vector.tensor_reduce(
                out=sumc, in_=psum_stats[:, :, :].transpose([0, 2, 1]),
                axis=AX.X, op=OP.add)
            nc.scalar.mul(out=sumc, in_=sumc,
                          mul=1.0 / float(cfg.D_TOT * H * W))
            # fold acc_max [P, W2, C] over W2 by in-place halving (2x TTs)
            wfold = W2
            while wfold > 1:
                wfold //= 2
                nc.vector.tensor_tensor(
                    out=acc_max[:, 0:wfold, :].rearrange("p w c -> p (w c)"),
                    in0=acc_max[:, 0:wfold, :].rearrange("p w c -> p (w c)"),
                    in1=acc_max[:, wfold:2 * wfold, :]
                    .rearrange("p w c -> p (w c)"),
                    op=OP.max)
            maxr = workp.tile([P, C], F32, tag="maxr", bufs=1)
            nc.gpsimd.partition_all_reduce(
                out_ap=maxr, in_ap=acc_max[:, 0, :], channels=P,
                reduce_op=bass_isa.ReduceOp.max)

            snd = dram.tile([2, C], F32, tag="snd")
            rcv = dram.tile([2, 2, C], F32, tag="rcv")
            nc.sync.dma_start(out=snd[0:1, :], in_=sumc)
            nc.sync.dma_start(out=snd[1:2, :], in_=maxr[0:1, :])
            if cfg.use_collectives:
                nc.gpsimd.collective_compute(
                    "AllGather", OP.bypass, replica_groups=groups,
                    ins=[snd.opt()], outs=[rcv.opt()])
            else:
                nc.gpsimd.dma_start(out=rcv[0], in_=snd)
                nc.gpsimd.dma_start(out=rcv[1], in_=snd)

            # ---------------- MLP -> ca (transpose-free) ----------------
            if cfg.stop_after == "pass1":
                return nc
            # land stats transposed: quadT[c, k, r] = rcv[r, k, c]
            quadT = workp.tile([C, 2, 2], F32, tag="quadT", bufs=1)
            for r in range(2):
                nc.sync.dma_start(out=quadT[:, :, r],
                                  in_=rcv[r].rearrange("k c -> c k"))
            pooled2 = workp.tile([C, 2], F32, tag="pooled2", bufs=1)
            nc.vector.tensor_add(out=pooled2[:, 0:1], in0=quadT[:, 0, 0:1],
                                 in1=quadT[:, 0, 1:2])
            nc.vector.tensor_tensor(out=pooled2[:, 1:2], in0=quadT[:, 1, 0:1],
                                    in1=quadT[:, 1, 1:2], op=OP.max)

            psum_h = ps_sm.tile([HID, 2], F32, tag="small")
            nc.tensor.matmul(out=psum_h, lhsT=w1_sb, rhs=pooled2,
                             start=True, stop=True)
            h2 = workp.tile([HID, 2], F32, tag="h2", bufs=1)
            nc.scalar.activation(out=h2, in_=psum_h, func=ACT.Relu,
                                 bias=b1t_sb)
            # psum_ca = h2.T @ w2 + 1x2.T @ b2 (bias folded in as a matmul)
            psum_ca = ps_sm.tile([2, C], F32, tag="small")
            nc.tensor.matmul(out=psum_ca, lhsT=h2, rhs=w2_sb,
                             start=True, stop=False)
            nc.tensor.matmul(out=psum_ca, lhsT=ones12, rhs=b2b[0:1, :],
                             start=False, stop=True)
            ca2 = workp.tile([2, C], BF16, tag="ca2", bufs=1)
            nc.scalar.activation(out=ca2, in_=psum_ca, func=ACT.Sigmoid)
            car = workp.tile([2, C], BF16, tag="car", bufs=1)
            nc.gpsimd.partition_all_reduce(
                out_ap=car, in_ap=ca2, channels=2,
                reduce_op=bass_isa.ReduceOp.add)
            ca_bf = consts.tile([P, C], BF16, tag="ca_bf")
            nc.gpsimd.partition_broadcast(out_ap=ca_bf, in_ap=car[0:1, :])

            # ---------------- phase 2: pooled + conv + apply ----------------
            if cfg.stop_after == "mlp":
                return nc

            def emit_pair_phase2a(j):
                """refine in-place; SUM tree to [P,W,4] + MAX tree to [P,W]
                (DVE, all 2x); perm matmuls (PE) fold the final 4-way sum
                via PSUM accumulation; psum->pooled copies (ACT)."""
                s_e, s_o = HALO + 2 * j, HALO + 2 * j + 1
                nc.vector.tensor_mul(
                    out=cache[j], in0=cache[j],
                    in1=_bc(ca_bf[:, :], [P, W, C], 1))
                # SUM tree: halve C 64 -> 4 (stays 2x throughout)
                t1s = treep.tile([P, W, C // 2], BF16, tag="t1add",
                                 name=f"t1add_{j}")
                with nc.allow_low_precision(reason="bf16 pooled stats"):
                    nc.vector.tensor_tensor(
                        out=t1s, in0=cache[j][:, :, 0:C // 2],
                        in1=cache[j][:, :, C // 2:], op=OP.add)
                    cf = C // 2
                    while cf > 4:
                        cf //= 2
                        nc.vector.tensor_tensor(
                            out=t1s[:, :, 0:cf], in0=t1s[:, :, 0:cf],
                            in1=t1s[:, :, cf:2 * cf], op=OP.add)
                # MAX tree: halve C 64 -> 1 (stays 2x throughout)
                t1m = treep.tile([P, W, C // 2], BF16, tag="t1max",
                                 name=f"t1max_{j}")
                rpm = workp.tile([P, W], BF16, tag="rpmax",
                                 name=f"rpmax_{j}")
                nc.vector.tensor_tensor(
                    out=t1m, in0=cache[j][:, :, 0:C // 2],
                    in1=cache[j][:, :, C // 2:], op=OP.max)
                cf = C // 2
                while cf > 4:
                    cf //= 2
                    nc.vector.tensor_tensor(
                        out=t1m[:, :, 0:cf], in0=t1m[:, :, 0:cf],
                        in1=t1m[:, :, cf:2 * cf], op=OP.max)
                nc.vector.tensor_reduce(
                    out=rpm, in_=t1m[:, :, 0:4], axis=AX.X, op=OP.max)
                # perm matmuls; the 4 leftover sum groups accumulate in PSUM
                for qa, qb, slot, nm in ((qa_e, qb_e, s_e, "pe"),
                                         (qa_o, qb_o, s_o, "po")):
                    pp = ps_perm.tile([P, W], F32, tag="perm",
                                      name=f"{nm}{j}")
                    for k in range(4):
                        nc.tensor.matmul(out=pp, lhsT=qa,
                                         rhs=t1s[:, :, k],
                                         start=(k == 0), stop=False)
                    nc.tensor.matmul(out=pp, lhsT=qb, rhs=rpm,
                                     start=False, stop=True)
                    nc.scalar.copy(out=pooled[:, slot, HALO:HALO + W], in_=pp)

            # edge pairs first: they feed the pooled-halo exchange
            pair_order = [0, 1, PAIRS - 2, PAIRS - 1] + list(range(2, PAIRS - 2))
            emitted = 0
            while emitted < 4:
                emit_pair_phase2a(pair_order[emitted])
                emitted += 1

            # ---- pooled-map halo exchange (pair-wise) ----
            snd_h = dram.tile([P, 6 * WP], BF16, tag="snd_h")
            rcv_h = dram.tile([2, P, 6 * WP], BF16, tag="rcv_h")
            nc.sync.dma_start(
                out=snd_h[:, 0:3 * WP],
                in_=pooled[:, HALO:2 * HALO, :].rearrange("p s w -> p (s w)"))
            nc.sync.dma_start(
                out=snd_h[:, 3 * WP:],
                in_=pooled[:, S - 2 * HALO:S - HALO, :]
                .rearrange("p s w -> p (s w)"))
            if cfg.use_collectives:
                nc.gpsimd.collective_compute(
                    "AllGather", OP.bypass, replica_groups=groups,
                    ins=[snd_h.opt()], outs=[rcv_h.opt()])
            else:
                nc.gpsimd.dma_start(out=rcv_h[0], in_=snd_h)
                nc.gpsimd.dma_start(out=rcv_h[1], in_=snd_h)
            par = nc.sync.partition_id() & 1
            # half 0: my top halo slots <- neighbor's first 3 own planes
            nc.sync.dma_start(
                out=pooled[:, S - HALO:S, :].rearrange("p s w -> p (s w)"),
                in_=rcv_h[1, :, 0:3 * WP], cond=1 - par)
            # half 1: my low halo slots <- neighbor's last 3 own planes
            nc.sync.dma_start(
                out=pooled[:, 0:HALO, :].rearrange("p s w -> p (s w)"),
                in_=rcv_h[0, :, 3 * WP:], cond=par)

            def emit_conv_blk(blk):
                pcv = ps_cv.tile([H, BLK, W], F32, tag="cv", name=f"cv{blk}")
                k = 0
                for kd in range(KS):
                    for kw in range(KS):
                        nc.tensor.matmul(
                            out=pcv,
                            lhsT=sband_sb[:, kd * KS + kw, :],
                            rhs=pooled[:, blk * BLK + kd: blk * BLK + kd + BLK,
                                       kw:kw + W],
                            start=(k == 0), stop=(k == NT - 1),
                            skip_group_check=True)
                        k += 1
                nc.scalar.activation(out=sa_sb[blk], in_=pcv, func=ACT.Sigmoid)
                sa_ev = sa_sb[blk].rearrange("h (a b) w -> h a b w", b=2)
                psp = ps_psp.tile([P, PPB, W], F32, tag="psp",
                                  name=f"psp{blk}")
                nc.tensor.matmul(out=psp, lhsT=qa_e[0:H, :],
                                 rhs=sa_ev[:, :, 0, :], start=True, stop=False)
                nc.tensor.matmul(out=psp, lhsT=qb_e[0:H, :],
                                 rhs=sa_ev[:, :, 1, :], start=False, stop=True)
                # duplicate along a trailing len-2 axis while copying out
                nc.scalar.copy(
                    out=sa_dup[blk],
                    in_=_bc(psp, [P, PPB, W, 2], 3))

            need_emit = [8, 12, 16, 16]   # pairs done before conv blk (order!)
            for blk in range(NB):
                while emitted < need_emit[blk]:
                    emit_pair_phase2a(pair_order[emitted])
                    emitted += 1
                emit_conv_blk(blk)
                for j in range(blk * PPB, blk * PPB + PPB):
                    dp = j - blk * PPB
                    cv = cache[j].rearrange("p w (a b) -> p w a b", b=2)
                    nc.vector.tensor_tensor(
                        out=cv, in0=cv,
                        in1=_bc(sa_dup[blk][:, dp], [P, W, C // 2, 2], 2),
                        op=OP.mult)
                    nc.sync.dma_start(
                        out=out_t[2 * j:2 * j + 2]
                        .rearrange("d h w c -> (d h) (w c)"),
                        in_=cache[j].rearrange("p w c -> p (w c)"))
    return nc


def make_sband(conv_w, cfg: Cfg):
    """Host-side band-matrix construction: [P, KS*KS, H] bf16.

    sband[ci*H+h', kd*KS+kw, h] = conv_w[kd, h'-h+halo, kw, ci] (avg rows
    pre-scaled by 1/C because the pooled map stores channel sums)."""
    H, C, KS, HALO = cfg.H, cfg.C, cfg.KS, cfg.HALO
    cw = np.asarray(conv_w, np.float32)[..., 0]        # [KS,KS,KS,2]
    sb = np.zeros((cfg.P, KS * KS, H), np.float32)
    h = np.arange(H)
    for kd in range(KS):
        for kw in range(KS):
            for ci in range(2):
                scale = (1.0 / C) if ci == 0 else 1.0
                for kh in range(KS):
                    hp = h + kh - HALO                  # h' = h + kh - halo
                    m = (hp >= 0) & (hp < H)
                    sb[ci * H + hp[m], kd * KS + kw, h[m]] = cw[kd, kh, kw, ci] * scale
    return sb.astype(ml_dtypes.bfloat16)


def make_core_inputs(x, w1, b1, w2, b2, sband_np, cfg: Cfg):
    """Shard the full inputs into per-core in_maps (no halo padding)."""
    C, D_LOC = cfg.C, cfg.D_LOC
    x = np.ascontiguousarray(np.asarray(x, np.float32))
    in_maps = []
    for core in range(cfg.N_CORES):
        b, half = core // 2, core % 2
        d0 = half * D_LOC
        in_maps.append({
            "xs": x[b, d0:d0 + D_LOC],
            "w1": np.asarray(w1, np.float32).reshape(C, cfg.HID),
            "b1t": np.asarray(b1, np.float32).reshape(cfg.HID, 1),
            "w2": np.asarray(w2, np.float32).reshape(cfg.HID, C),
            "b2": np.asarray(b2, np.float32).reshape(1, C),
            "sband": sband_np,
        })
    return in_maps


_COMPILED = {}


def get_compiled(cfg: Cfg = FULL):
    if cfg not in _COMPILED:
        nc = bacc.Bacc("TRN2", target_bir_lowering=False, debug=False,
                       num_devices=cfg.N_CORES)
        build_cbam(nc, cfg)
        nc.compile()
        _COMPILED[cfg] = nc
    return _COMPILED[cfg]


def kernel(x, w1, b1, w2, b2, conv_w):
    from concourse.bass_utils import run_bass_kernel_spmd

    cfg = FULL
    nc = get_compiled(cfg)
    sband_np = make_sband(conv_w, cfg)
    in_maps = make_core_inputs(x, w1, b1, w2, b2, sband_np, cfg)
    res = run_bass_kernel_spmd(nc, in_maps, list(range(cfg.N_CORES)))
    B, D = 4, 64
    out = np.empty((B, D, cfg.H, cfg.W, cfg.C), np.float32)
    for core in range(cfg.N_CORES):
        b, half = core // 2, core % 2
        d0 = half * cfg.D_LOC
        out[b, d0:d0 + cfg.D_LOC] = np.asarray(
            res.results[core]["out"], dtype=np.float32)
    return out


# revision 20
# speedup vs baseline: 1.0865x; 1.0865x over previous
"""CBAM3D Trainium2 kernel (8 NeuronCores, SPMD).

Reference computation (per batch sample b):
  avg_pool[c] = mean_{d,h,w} x ; max_pool[c] = max_{d,h,w} x
  ca = sigmoid(relu(avg@w1+b1)@w2+b2) + sigmoid(relu(max@w1+b1)@w2+b2)
  refined = x * ca[c]
  P = [mean_c refined, max_c refined]            # [D,H,W,2]
  sa = sigmoid(conv3d_same(P, conv_w))           # 7x7x7x2 -> 1
  out = refined * sa

Sharding: core i handles sample b=i//2, D-half half=i%2 (32 planes, NO host
halo padding). Cross-core traffic: a pair-wise AllGather of channel sum/max
stats (512B) and a pair-wise AllGather of the 3-slot pooled-map halo
(~108KB) — the full-resolution x halo is never re-read from HBM.

Per-core pipeline (engine balance is the whole game; DVE is the wall):
  pass1: stream x f32 (plane-pair tiles), cast to a bf16 SBUF cache on the
         Scalar engine, accumulate channel sum (PE matmul vs ones) and
         channel max (running elementwise max on DVE, 2x)
  AllGather stats over {2i,2i+1}; transpose-free tiny MLP on device -> ca
  phase2a per pair: refined = cache*ca in-place (DVE 2x); channel SUM and
         MAX trees (DVE 2x + one 1x reduce each); pooled map
         [(ci,h'), slot, w+pad] via permutation matmuls (PE) + psum->pooled
         copies (ACT). Edge pairs go first; then the pooled halo exchange
         (AllGather + parity-conditional DMAs into the halo slots).
  conv:  49 taps x 4 blocks (8 planes) of accumulating matmuls with
         host-prebuilt band matrices (kh,ci folded into K=128) -> sigmoid
         -> sa stored C-pair-duplicated so the apply runs at DVE 2x
         (a stride-0 broadcast would force 1x — measured 2.2x slower)
  apply: cache *= sa in-place (DVE 2x), one DMA per plane-pair to HBM bf16
"""

from dataclasses import dataclass

import numpy as np
import ml_dtypes

import concourse.bass as bass
import concourse.tile as tile
import concourse.mybir as mybir
from concourse import bacc, bass_isa

F32 = mybir.dt.float32
BF16 = mybir.dt.bfloat16
AX = mybir.AxisListType
OP = mybir.AluOpType
ACT = mybir.ActivationFunctionType


@dataclass(frozen=True)
class Cfg:
    H: int = 64
    W: int = 64
    C: int = 64
    D_LOC: int = 32          # own planes per core
    HID: int = 4             # C // reduction_ratio
    KS: int = 7
    N_CORES: int = 8
    use_collectives: bool = True
    stop_after: str = "full"   # pass1 | mlp | full

    @property
    def HALO(self):
        return self.KS // 2

    @property
    def S(self):
        return self.D_LOC + 2 * self.HALO   # slots in the pooled map

    @property
    def P(self):
        return 2 * self.H                    # partition dim of pair tiles

    @property
    def WP(self):
        return self.W + 2 * self.HALO        # padded pooled-map width

    @property
    def D_TOT(self):
        return 2 * self.D_LOC                # full-sample depth (2 shards)


FULL = Cfg()


def _bc(ap, shape, axis):
    """broadcast ap (by unsqueezing `axis`) to `shape`"""
    return ap.unsqueeze(axis).broadcast_to(shape)


def build_cbam(nc, cfg: Cfg):
    H, W, C = cfg.H, cfg.W, cfg.C
    P, S, WP, HALO = cfg.P, cfg.S, cfg.WP, cfg.HALO
    D_LOC, HID, KS = cfg.D_LOC, cfg.HID, cfg.KS
    PAIRS = D_LOC // 2
    BLK = 8                                  # d-planes per conv block
    NB = D_LOC // BLK
    PPB = BLK // 2                           # plane-pairs per conv block
    W2 = W // 2
    NT = KS * KS

    xs = nc.dram_tensor("xs", [D_LOC, H, W, C], F32, kind="ExternalInput").ap()
    w1 = nc.dram_tensor("w1", [C, HID], F32, kind="ExternalInput").ap()
    b1t = nc.dram_tensor("b1t", [HID, 1], F32, kind="ExternalInput").ap()
    w2 = nc.dram_tensor("w2", [HID, C], F32, kind="ExternalInput").ap()
    b2 = nc.dram_tensor("b2", [1, C], F32, kind="ExternalInput").ap()
    sband = nc.dram_tensor("sband", [P, NT, H], BF16, kind="ExternalInput").ap()
    out_t = nc.dram_tensor("out", [D_LOC, H, W, C], BF16, kind="ExternalOutput").ap()

    groups = [[i, i + 1] for i in range(0, cfg.N_CORES, 2)]

    with tile.TileContext(nc) as tc:
        with (
            tc.tile_pool(name="consts", bufs=1) as consts,
            tc.tile_pool(name="cache", bufs=1) as cachep,
            tc.tile_pool(name="stage", bufs=5) as stagep,
            tc.tile_pool(name="tree", bufs=1) as treep,
            tc.tile_pool(name="work", bufs=2) as workp,
            tc.tile_pool(name="dram", bufs=1, space="DRAM") as dram,
            tc.tile_pool(name="ps_stats", bufs=1, space="PSUM") as ps_stats,
            tc.tile_pool(name="ps_perm", bufs=2, space="PSUM") as ps_perm,
            tc.tile_pool(name="ps_psp", bufs=2, space="PSUM") as ps_psp,
            tc.tile_pool(name="ps_cv", bufs=2, space="PSUM") as ps_cv,
            tc.tile_pool(name="ps_sm", bufs=1, space="PSUM") as ps_sm,
        ):
            # ---------------- constants ----------------
            ones = consts.tile([P, 1], BF16, tag="ones")
            nc.vector.memset(ones, 1.0)

            # bf16 permutation matrices; pooled partition layout is (ci*H+h').
            def diag(t, col_lo, col_hi, base):
                nc.gpsimd.affine_select(
                    out=t[:, col_lo:col_hi], in_=t[:, col_lo:col_hi],
                    compare_op=OP.not_equal, fill=1.0, base=base,
                    pattern=[[-1, col_hi - col_lo]], channel_multiplier=1)

            qa_e = consts.tile([P, P], BF16, tag="qa_e")
            qb_e = consts.tile([P, P], BF16, tag="qb_e")
            qa_o = consts.tile([P, P], BF16, tag="qa_o")
            qb_o = consts.tile([P, P], BF16, tag="qb_o")
            for t in (qa_e, qb_e, qa_o, qb_o):
                nc.gpsimd.memset(t, 0.0)
            diag(qa_e, 0, H, 0)
            diag(qb_e, H, P, 0)
            diag(qa_o, 0, H, -H)
            diag(qb_o, H, P, -H)

            sband_sb = consts.tile([P, NT, H], BF16, tag="sband")
            nc.sync.dma_start(
                out=sband_sb[:].rearrange("p t h -> p (t h)"),
                in_=sband.rearrange("p t h -> p (t h)"))
            w1_sb = consts.tile([C, HID], F32, tag="w1")
            nc.sync.dma_start(out=w1_sb, in_=w1)
            w2_sb = consts.tile([HID, C], F32, tag="w2")
            nc.sync.dma_start(out=w2_sb, in_=w2)
            b1t_sb = consts.tile([HID, 1], F32, tag="b1t")
            nc.sync.dma_start(out=b1t_sb, in_=b1t)

            def dma_bcast(dst, src_ap, parts):
                a = bass.AP(tensor=src_ap.tensor, offset=src_ap.offset,
                            ap=[[0, parts]] + [list(p) for p in src_ap.ap[1:]])
                nc.sync.dma_start(out=dst, in_=a)

            b2b = consts.tile([2, C], F32, tag="b2")
            dma_bcast(b2b, b2, 2)

            # pre-warm the ACT table set (Relu/Sigmoid) so the first real
            # activation in the latency-critical MLP doesn't pay the load
            warm = consts.tile([1, 1], F32, tag="warm")
            nc.scalar.activation(out=warm, in_=b2b[0:1, 0:1], func=ACT.Relu)
            nc.scalar.activation(out=warm, in_=warm, func=ACT.Sigmoid)
            ones12 = consts.tile([1, 2], F32, tag="ones12")
            nc.vector.memset(ones12, 1.0)

            if cfg.use_collectives:
                wu_s = dram.tile([1, 1], F32, tag="wu_s")
                wu_r = dram.tile([2, 1], F32, tag="wu_r")
                nc.gpsimd.dma_start(out=wu_s, in_=b2b[0:1, 0:1])
                nc.gpsimd.collective_compute(
                    "AllGather", OP.bypass, replica_groups=groups,
                    ins=[wu_s.opt()], outs=[wu_r.opt()])

            # persistent state. pair j covers planes (2j, 2j+1) -> pooled
            # slots (HALO+2j, HALO+2j+1). Halo slots 0:3 / 35:38 come from
            # the neighbor core (or stay zero at sample boundaries).
            cache = [cachep.tile([P, W, C], BF16, tag=f"cache{j}",
                                 name=f"cache{j}") for j in range(PAIRS)]
            acc_max = cachep.tile([P, W2, C], BF16, tag="acc_max")
            nc.vector.memset(acc_max, -3.0e38)
            pooled = cachep.tile([P, S, WP], BF16, tag="pooled")
            nc.gpsimd.memset(pooled, 0.0)
            sa_sb = [cachep.tile([H, BLK, W], BF16, tag=f"sa{b}", name=f"sa{b}")
                     for b in range(NB)]
            # sa duplicated along a trailing len-2 axis: the apply
            # tensor_tensor then reads packed bf16 pairs (2x DVE mode).
            sa_dup = [cachep.tile([P, PPB, W, 2], BF16, tag=f"sad{b}",
                                  name=f"sad{b}") for b in range(NB)]

            # ---------------- pass 1: stream + cast + stats ----------------
            # (HWDGE f32 loads + ACT casts; a casting SWDGE DMA was tried
            # and runs ~30% below line rate — the ACT cast hides fully)
            psum_stats = ps_stats.tile([1, 8, C], F32, tag="stats")
            n_wg = W // 8
            mm_i = 0
            n_mm = PAIRS * n_wg
            for j in range(PAIRS):
                for wh in range(2):
                    st = stagep.tile([P, W2, C], F32, tag="stage")
                    nc.sync.dma_start(
                        out=st.rearrange("p w c -> p (w c)"),
                        in_=xs[2 * j:2 * j + 2, :, wh * W2:(wh + 1) * W2, :]
                        .rearrange("d h w c -> (d h) (w c)"))
                    nc.scalar.copy(
                        out=cache[j][:, wh * W2:(wh + 1) * W2, :], in_=st)
                    # channel max: running elementwise max over half tiles
                    nc.vector.tensor_tensor(
                        out=acc_max[:].rearrange("p w c -> p (w c)"),
                        in0=acc_max[:].rearrange("p w c -> p (w c)"),
                        in1=cache[j][:, wh * W2:(wh + 1) * W2, :]
                        .rearrange("p w c -> p (w c)"),
                        op=OP.max)
                for g in range(n_wg):
                    nc.tensor.matmul(
                        out=psum_stats,
                        lhsT=ones[:, :],
                        rhs=cache[j][:, g * 8:(g + 1) * 8, :],
                        start=(mm_i == 0), stop=(mm_i == n_mm - 1))
                    mm_i += 1

            # finalize stats (mean scale applied here, off the critical path)
            sumc = workp.tile([1, C], F32, tag="sumc", bufs=1)
            nc.vector.tensor_reduce(
                out=sumc, in_=psum_stats[:, :, :].transpose([0, 2, 1]),
                axis=AX.X, op=OP.add)
            nc.scalar.mul(out=sumc, in_=sumc,
                          mul=1.0 / float(cfg.D_TOT * H * W))
            # fold acc_max [P, W2, C] over W2 by in-place halving (2x TTs)
            wfold = W2
            while wfold > 1:
                wfold //= 2
                nc.vector.tensor_tensor(
                    out=acc_max[:, 0:wfold, :].rearrange("p w c -> p (w c)"),
                    in0=acc_max[:, 0:wfold, :].rearrange("p w c -> p (w c)"),
                    in1=acc_max[:, wfold:2 * wfold, :]
                    .rearrange("p w c -> p (w c)"),
                    op=OP.max)
            maxr = workp.tile([P, C], F32, tag="maxr", bufs=1)
            nc.gpsimd.partition_all_reduce(
                out_ap=maxr, in_ap=acc_max[:, 0, :], channels=P,
                reduce_op=bass_isa.ReduceOp.max)

            snd = dram.tile([2, C], F32, tag="snd")
            rcv = dram.tile([2, 2, C], F32, tag="rcv")
            nc.sync.dma_start(out=snd[0:1, :], in_=sumc)
            nc.sync.dma_start(out=snd[1:2, :], in_=maxr[0:1, :])
            if cfg.use_collectives:
                nc.gpsimd.collective_compute(
                    "AllGather", OP.bypass, replica_groups=groups,
                    ins=[snd.opt()], outs=[rcv.opt()])
            else:
                nc.gpsimd.dma_start(out=rcv[0], in_=snd)
                nc.gpsimd.dma_start(out=rcv[1], in_=snd)

            # ---------------- MLP -> ca (transpose-free) ----------------
            if cfg.stop_after == "pass1":
                return nc
            # land stats transposed: quadT[c, k, r] = rcv[r, k, c]
            quadT = workp.tile([C, 2, 2], F32, tag="quadT", bufs=1)
            for r in range(2):
                nc.sync.dma_start(out=quadT[:, :, r],
                                  in_=rcv[r].rearrange("k c -> c k"))
            pooled2 = workp.tile([C, 2], F32, tag="pooled2", bufs=1)
            nc.vector.tensor_add(out=pooled2[:, 0:1], in0=quadT[:, 0, 0:1],
                                 in1=quadT[:, 0, 1:2])
            nc.vector.tensor_tensor(out=pooled2[:, 1:2], in0=quadT[:, 1, 0:1],
                                    in1=quadT[:, 1, 1:2], op=OP.max)

            psum_h = ps_sm.tile([HID, 2], F32, tag="small")
            nc.tensor.matmul(out=psum_h, lhsT=w1_sb, rhs=pooled2,
                             start=True, stop=True)
            h2 = workp.tile([HID, 2], F32, tag="h2", bufs=1)
            nc.scalar.activation(out=h2, in_=psum_h, func=ACT.Relu,
                                 bias=b1t_sb)
            # psum_ca = h2.T @ w2 + 1x2.T @ b2 (bias folded in as a matmul)
            psum_ca = ps_sm.tile([2, C], F32, tag="small")
            nc.tensor.matmul(out=psum_ca, lhsT=h2, rhs=w2_sb,
                             start=True, stop=False)
            nc.tensor.matmul(out=psum_ca, lhsT=ones12, rhs=b2b[0:1, :],
                             start=False, stop=True)
            ca2 = workp.tile([2, C], BF16, tag="ca2", bufs=1)
            nc.scalar.activation(out=ca2, in_=psum_ca, func=ACT.Sigmoid)
            car = workp.tile([2, C], BF16, tag="car", bufs=1)
            nc.gpsimd.partition_all_reduce(
                out_ap=car, in_ap=ca2, channels=2,
                reduce_op=bass_isa.ReduceOp.add)
            ca_bf = consts.tile([P, C], BF16, tag="ca_bf")
            nc.gpsimd.partition_broadcast(out_ap=ca_bf, in_ap=car[0:1, :])

            # ---------------- phase 2: pooled + conv + apply ----------------
            if cfg.stop_after == "mlp":
                return nc

            def emit_pair_phase2a(j):
                """refine in-place; SUM tree to [P,W,4] + MAX tree to [P,W]
                (DVE, all 2x); perm matmuls (PE) fold the final 4-way sum
                via PSUM accumulation; psum->pooled copies (ACT)."""
                s_e, s_o = HALO + 2 * j, HALO + 2 * j + 1
                nc.vector.tensor_mul(
                    out=cache[j], in0=cache[j],
                    in1=_bc(ca_bf[:, :], [P, W, C], 1))
                # SUM tree: halve C 64 -> 4 (stays 2x throughout)
                t1s = treep.tile([P, W, C // 2], BF16, tag="t1add",
                                 name=f"t1add_{j}")
                with nc.allow_low_precision(reason="bf16 pooled stats"):
                    nc.vector.tensor_tensor(
                        out=t1s, in0=cache[j][:, :, 0:C // 2],
                        in1=cache[j][:, :, C // 2:], op=OP.add)
                    cf = C // 2
                    while cf > 4:
                        cf //= 2
                        nc.vector.tensor_tensor(
                            out=t1s[:, :, 0:cf], in0=t1s[:, :, 0:cf],
                            in1=t1s[:, :, cf:2 * cf], op=OP.add)
                # MAX tree: halve C 64 -> 1 (stays 2x throughout)
                t1m = treep.tile([P, W, C // 2], BF16, tag="t1max",
                                 name=f"t1max_{j}")
                rpm = workp.tile([P, W], BF16, tag="rpmax",
                                 name=f"rpmax_{j}")
                nc.vector.tensor_tensor(
                    out=t1m, in0=cache[j][:, :, 0:C // 2],
                    in1=cache[j][:, :, C // 2:], op=OP.max)
                cf = C // 2
                while cf > 4:
                    cf //= 2
                    nc.vector.tensor_tensor(
                        out=t1m[:, :, 0:cf], in0=t1m[:, :, 0:cf],
                        in1=t1m[:, :, cf:2 * cf], op=OP.max)
                nc.vector.tensor_reduce(
                    out=rpm, in_=t1m[:, :, 0:4], axis=AX.X, op=OP.max)
                # perm matmuls; the 4 leftover sum groups accumulate in PSUM
                for qa, qb, slot, nm in ((qa_e, qb_e, s_e, "pe"),
                                         (qa_o, qb_o, s_o, "po")):
                    pp = ps_perm.tile([P, W], F32, tag="perm",
                                      name=f"{nm}{j}")
                    for k in range(4):
                        nc.tensor.matmul(out=pp, lhsT=qa,
                                         rhs=t1s[:, :, k],
                                         start=(k == 0), stop=False)
                    nc.tensor.matmul(out=pp, lhsT=qb, rhs=rpm,
                                     start=False, stop=True)
                    nc.scalar.copy(out=pooled[:, slot, HALO:HALO + W], in_=pp)

            # edge pairs first: they feed the pooled-halo exchange
            pair_order = [0, 1, PAIRS - 2, PAIRS - 1] + list(range(2, PAIRS - 2))
            emitted = 0
            while emitted < 4:
                emit_pair_phase2a(pair_order[emitted])
                emitted += 1

            # ---- pooled-map halo exchange (pair-wise) ----
            snd_h = dram.tile([P, 6 * WP], BF16, tag="snd_h")
            rcv_h = dram.tile([2, P, 6 * WP], BF16, tag="rcv_h")
            nc.sync.dma_start(
                out=snd_h[:, 0:3 * WP],
                in_=pooled[:, HALO:2 * HALO, :].rearrange("p s w -> p (s w)"))
            nc.sync.dma_start(
                out=snd_h[:, 3 * WP:],
                in_=pooled[:, S - 2 * HALO:S - HALO, :]
                .rearrange("p s w -> p (s w)"))
            if cfg.use_collectives:
                nc.gpsimd.collective_compute(
                    "AllGather", OP.bypass, replica_groups=groups,
                    ins=[snd_h.opt()], outs=[rcv_h.opt()])
            else:
                nc.gpsimd.dma_start(out=rcv_h[0], in_=snd_h)
                nc.gpsimd.dma_start(out=rcv_h[1], in_=snd_h)
            par = nc.sync.partition_id() & 1
            # half 0: my top halo slots <- neighbor's first 3 own planes
            nc.sync.dma_start(
                out=pooled[:, S - HALO:S, :].rearrange("p s w -> p (s w)"),
                in_=rcv_h[1, :, 0:3 * WP], cond=1 - par)
            # half 1: my low halo slots <- neighbor's last 3 own planes
            nc.sync.dma_start(
                out=pooled[:, 0:HALO, :].rearrange("p s w -> p (s w)"),
                in_=rcv_h[0, :, 3 * WP:], cond=par)

            def emit_conv_blk(blk):
                pcv = ps_cv.tile([H, BLK, W], F32, tag="cv", name=f"cv{blk}")
                k = 0
                for kd in range(KS):
                    for kw in range(KS):
                        nc.tensor.matmul(
                            out=pcv,
                            lhsT=sband_sb[:, kd * KS + kw, :],
                            rhs=pooled[:, blk * BLK + kd: blk * BLK + kd + BLK,
                                       kw:kw + W],
                            start=(k == 0), stop=(k == NT - 1),
                            skip_group_check=True)
                        k += 1
                nc.scalar.activation(out=sa_sb[blk], in_=pcv, func=ACT.Sigmoid)
                sa_ev = sa_sb[blk].rearrange("h (a b) w -> h a b w", b=2)
                psp = ps_psp.tile([P, PPB, W], F32, tag="psp",
                                  name=f"psp{blk}")
                nc.tensor.matmul(out=psp, lhsT=qa_e[0:H, :],
                                 rhs=sa_ev[:, :, 0, :], start=True, stop=False)
                nc.tensor.matmul(out=psp, lhsT=qb_e[0:H, :],
                                 rhs=sa_ev[:, :, 1, :], start=False, stop=True)
                # duplicate along a trailing len-2 axis while copying out
                nc.scalar.copy(
                    out=sa_dup[blk],
                    in_=_bc(psp, [P, PPB, W, 2], 3))

            need_emit = [8, 12, 16, 16]   # pairs done before conv blk (order!)
            for blk in range(NB):
                while emitted < need_emit[blk]:
                    emit_pair_phase2a(pair_order[emitted])
                    emitted += 1
                emit_conv_blk(blk)
                for j in range(blk * PPB, blk * PPB + PPB):
                    dp = j - blk * PPB
                    cv = cache[j].rearrange("p w (a b) -> p w a b", b=2)
                    nc.vector.tensor_tensor(
                        out=cv, in0=cv,
                        in1=_bc(sa_dup[blk][:, dp], [P, W, C // 2, 2], 2),
                        op=OP.mult)
                    nc.sync.dma_start(
                        out=out_t[2 * j:2 * j + 2]
                        .rearrange("d h w c -> (d h) (w c)"),
                        in_=cache[j].rearrange("p w c -> p (w c)"))
    return nc


def make_sband(conv_w, cfg: Cfg):
    """Host-side band-matrix construction: [P, KS*KS, H] bf16.

    sband[ci*H+h', kd*KS+kw, h] = conv_w[kd, h'-h+halo, kw, ci] (avg rows
    pre-scaled by 1/C because the pooled map stores channel sums)."""
    H, C, KS, HALO = cfg.H, cfg.C, cfg.KS, cfg.HALO
    cw = np.asarray(conv_w, np.float32)[..., 0]        # [KS,KS,KS,2]
    sb = np.zeros((cfg.P, KS * KS, H), np.float32)
    h = np.arange(H)
    for kd in range(KS):
        for kw in range(KS):
            for ci in range(2):
                scale = (1.0 / C) if ci == 0 else 1.0
                for kh in range(KS):
                    hp = h + kh - HALO                  # h' = h + kh - halo
                    m = (hp >= 0) & (hp < H)
                    sb[ci * H + hp[m], kd * KS + kw, h[m]] = cw[kd, kh, kw, ci] * scale
    return sb.astype(ml_dtypes.bfloat16)


def make_core_inputs(x, w1, b1, w2, b2, sband_np, cfg: Cfg):
    """Shard the full inputs into per-core in_maps (no halo padding)."""
    C, D_LOC = cfg.C, cfg.D_LOC
    x = np.ascontiguousarray(np.asarray(x, np.float32))
    in_maps = []
    for core in range(cfg.N_CORES):
        b, half = core // 2, core % 2
        d0 = half * D_LOC
        in_maps.append({
            "xs": x[b, d0:d0 + D_LOC],
            "w1": np.asarray(w1, np.float32).reshape(C, cfg.HID),
            "b1t": np.asarray(b1, np.float32).reshape(cfg.HID, 1),
            "w2": np.asarray(w2, np.float32).reshape(cfg.HID, C),
            "b2": np.asarray(b2, np.float32).reshape(1, C),
            "sband": sband_np,
        })
    return in_maps


_COMPILED = {}


def get_compiled(cfg: Cfg = FULL):
    if cfg not in _COMPILED:
        nc = bacc.Bacc("TRN2", target_bir_lowering=False, debug=False,
                       num_devices=cfg.N_CORES)
        build_cbam(nc, cfg)
        nc.compile()
        _COMPILED[cfg] = nc
    return _COMPILED[cfg]


def kernel(x, w1, b1, w2, b2, conv_w):
    from concourse.bass_utils import run_bass_kernel_spmd

    cfg = FULL
    nc = get_compiled(cfg)
    sband_np = make_sband(conv_w, cfg)
    in_maps = make_core_inputs(x, w1, b1, w2, b2, sband_np, cfg)
    res = run_bass_kernel_spmd(nc, in_maps, list(range(cfg.N_CORES)))
    B, D = 4, 64
    out = np.empty((B, D, cfg.H, cfg.W, cfg.C), np.float32)
    for core in range(cfg.N_CORES):
        b, half = core // 2, core % 2
        d0 = half * cfg.D_LOC
        out[b, d0:d0 + cfg.D_LOC] = np.asarray(
            res.results[core]["out"], dtype=np.float32)
    return out


# revision 22
# speedup vs baseline: 1.1028x; 1.0149x over previous
"""CBAM3D Trainium2 kernel (8 NeuronCores, SPMD).

Reference computation (per batch sample b):
  avg_pool[c] = mean_{d,h,w} x ; max_pool[c] = max_{d,h,w} x
  ca = sigmoid(relu(avg@w1+b1)@w2+b2) + sigmoid(relu(max@w1+b1)@w2+b2)
  refined = x * ca[c]
  P = [mean_c refined, max_c refined]            # [D,H,W,2]
  sa = sigmoid(conv3d_same(P, conv_w))           # 7x7x7x2 -> 1
  out = refined * sa

Sharding: core i handles sample b=i//2, D-half half=i%2 (32 planes, NO host
halo padding). Cross-core traffic: a pair-wise AllGather of channel sum/max
stats (512B) and a pair-wise AllGather of the 3-slot pooled-map halo
(~108KB) — the full-resolution x halo is never re-read from HBM.

Per-core pipeline (engine balance is the whole game; DVE is the wall):
  pass1: stream x f32 (plane-pair tiles), cast to a bf16 SBUF cache on the
         Scalar engine, accumulate channel sum (PE matmul vs ones) and
         channel max (running elementwise max on DVE, 2x)
  AllGather stats over {2i,2i+1}; transpose-free tiny MLP on device -> ca
  phase2a per pair: refined = cache*ca in-place (DVE 2x); channel SUM and
         MAX trees (DVE 2x + one 1x reduce each); pooled map
         [(ci,h'), slot, w+pad] via permutation matmuls (PE) + psum->pooled
         copies (ACT). Edge pairs go first; then the pooled halo exchange
         (AllGather + parity-conditional DMAs into the halo slots).
  conv:  49 taps x 4 blocks (8 planes) of accumulating matmuls with
         host-prebuilt band matrices (kh,ci folded into K=128) -> sigmoid
         -> sa stored C-pair-duplicated so the apply runs at DVE 2x
         (a stride-0 broadcast would force 1x — measured 2.2x slower)
  apply: cache *= sa in-place (DVE 2x), one DMA per plane-pair to HBM bf16
"""

from dataclasses import dataclass

import numpy as np
import ml_dtypes

import concourse.bass as bass
import concourse.tile as tile
import concourse.mybir as mybir
from concourse import bacc, bass_isa

F32 = mybir.dt.float32
BF16 = mybir.dt.bfloat16
AX = mybir.AxisListType
OP = mybir.AluOpType
ACT = mybir.ActivationFunctionType


@dataclass(frozen=True)
class Cfg:
    H: int = 64
    W: int = 64
    C: int = 64
    D_LOC: int = 32          # own planes per core
    HID: int = 4             # C // reduction_ratio
    KS: int = 7
    N_CORES: int = 8
    use_collectives: bool = True
    stop_after: str = "full"   # pass1 | mlp | full

    @property
    def HALO(self):
        return self.KS // 2

    @property
    def S(self):
        return self.D_LOC + 2 * self.HALO   # slots in the pooled map

    @property
    def P(self):
        return 2 * self.H                    # partition dim of pair tiles

    @property
    def WP(self):
        return self.W + 2 * self.HALO        # padded pooled-map width

    @property
    def D_TOT(self):
        return 2 * self.D_LOC                # full-sample depth (2 shards)


FULL = Cfg()


def _bc(ap, shape, axis):
    """broadcast ap (by unsqueezing `axis`) to `shape`"""
    return ap.unsqueeze(axis).broadcast_to(shape)


def build_cbam(nc, cfg: Cfg):
    H, W, C = cfg.H, cfg.W, cfg.C
    P, S, WP, HALO = cfg.P, cfg.S, cfg.WP, cfg.HALO
    D_LOC, HID, KS = cfg.D_LOC, cfg.HID, cfg.KS
    PAIRS = D_LOC // 2
    BLK = 8                                  # d-planes per conv block
    NB = D_LOC // BLK
    PPB = BLK // 2                           # plane-pairs per conv block
    W2 = W // 2
    NT = KS * KS

    xs = nc.dram_tensor("xs", [D_LOC, H, W, C], F32, kind="ExternalInput").ap()
    w1 = nc.dram_tensor("w1", [C, HID], F32, kind="ExternalInput").ap()
    b1t = nc.dram_tensor("b1t", [HID, 1], F32, kind="ExternalInput").ap()
    w2 = nc.dram_tensor("w2", [HID, C], F32, kind="ExternalInput").ap()
    b2 = nc.dram_tensor("b2", [1, C], F32, kind="ExternalInput").ap()
    sband = nc.dram_tensor("sband", [P, NT, H], BF16, kind="ExternalInput").ap()
    out_t = nc.dram_tensor("out", [D_LOC, H, W, C], BF16, kind="ExternalOutput").ap()

    groups = [[i, i + 1] for i in range(0, cfg.N_CORES, 2)]

    with tile.TileContext(nc) as tc:
        with (
            tc.tile_pool(name="consts", bufs=1) as consts,
            tc.tile_pool(name="cache", bufs=1) as cachep,
            tc.tile_pool(name="stage", bufs=5) as stagep,
            tc.tile_pool(name="tree", bufs=1) as treep,
            tc.tile_pool(name="work", bufs=2) as workp,
            tc.tile_pool(name="dram", bufs=1, space="DRAM") as dram,
            tc.tile_pool(name="ps_stats", bufs=1, space="PSUM") as ps_stats,
            tc.tile_pool(name="ps_perm", bufs=2, space="PSUM") as ps_perm,
            tc.tile_pool(name="ps_psp", bufs=2, space="PSUM") as ps_psp,
            tc.tile_pool(name="ps_cv", bufs=2, space="PSUM") as ps_cv,
            tc.tile_pool(name="ps_sm", bufs=1, space="PSUM") as ps_sm,
        ):
            # ---------------- constants ----------------
            ones = consts.tile([P, 1], BF16, tag="ones")
            nc.vector.memset(ones, 1.0)

            # bf16 permutation matrices; pooled partition layout is (ci*H+h').
            def diag(t, col_lo, col_hi, base):
                nc.gpsimd.affine_select(
                    out=t[:, col_lo:col_hi], in_=t[:, col_lo:col_hi],
                    compare_op=OP.not_equal, fill=1.0, base=base,
                    pattern=[[-1, col_hi - col_lo]], channel_multiplier=1)

            qa_e = consts.tile([P, P], BF16, tag="qa_e")
            qb_e = consts.tile([P, P], BF16, tag="qb_e")
            qa_o = consts.tile([P, P], BF16, tag="qa_o")
            qb_o = consts.tile([P, P], BF16, tag="qb_o")
            for t in (qa_e, qb_e, qa_o, qb_o):
                nc.gpsimd.memset(t, 0.0)
            diag(qa_e, 0, H, 0)
            diag(qb_e, H, P, 0)
            diag(qa_o, 0, H, -H)
            diag(qb_o, H, P, -H)

            sband_sb = consts.tile([P, NT, H], BF16, tag="sband")
            nc.gpsimd.dma_start(
                out=sband_sb[:].rearrange("p t h -> p (t h)"),
                in_=sband.rearrange("p t h -> p (t h)"))
            w1_sb = consts.tile([C, HID], F32, tag="w1")
            nc.gpsimd.dma_start(out=w1_sb, in_=w1)
            w2_sb = consts.tile([HID, C], F32, tag="w2")
            nc.gpsimd.dma_start(out=w2_sb, in_=w2)
            b1t_sb = consts.tile([HID, 1], F32, tag="b1t")
            nc.gpsimd.dma_start(out=b1t_sb, in_=b1t)

            def dma_bcast(dst, src_ap, parts):
                a = bass.AP(tensor=src_ap.tensor, offset=src_ap.offset,
                            ap=[[0, parts]] + [list(p) for p in src_ap.ap[1:]])
                nc.gpsimd.dma_start(out=dst, in_=a)

            b2b = consts.tile([2, C], F32, tag="b2")
            dma_bcast(b2b, b2, 2)

            # pre-warm the ACT table set (Relu/Sigmoid) so the first real
            # activation in the latency-critical MLP doesn't pay the load
            warm = consts.tile([1, 1], F32, tag="warm")
            nc.scalar.activation(out=warm, in_=b2b[0:1, 0:1], func=ACT.Relu)
            nc.scalar.activation(out=warm, in_=warm, func=ACT.Sigmoid)
            ones12 = consts.tile([1, 2], F32, tag="ones12")
            nc.vector.memset(ones12, 1.0)

            if cfg.use_collectives:
                wu_s = dram.tile([1, 1], F32, tag="wu_s")
                wu_r = dram.tile([2, 1], F32, tag="wu_r")
                nc.gpsimd.dma_start(out=wu_s, in_=b2b[0:1, 0:1])
                nc.gpsimd.collective_compute(
                    "AllGather", OP.bypass, replica_groups=groups,
                    ins=[wu_s.opt()], outs=[wu_r.opt()])

            # persistent state. pair j covers planes (2j, 2j+1) -> pooled
            # slots (HALO+2j, HALO+2j+1). Halo slots 0:3 / 35:38 come from
            # the neighbor core (or stay zero at sample boundaries).
            cache = [cachep.tile([P, W, C], BF16, tag=f"cache{j}",
                                 name=f"cache{j}") for j in range(PAIRS)]
            acc_max = cachep.tile([P, W2, C], BF16, tag="acc_max")
            nc.vector.memset(acc_max, -3.0e38)
            pooled = cachep.tile([P, S, WP], BF16, tag="pooled")
            nc.gpsimd.memset(pooled, 0.0)
            # conv blocks (start plane, size): the final 8 planes are two
            # 4-plane blocks so the last tree->conv->apply tail is shorter
            conv_blocks = [(0, 8), (8, 8), (16, 8), (24, 4), (28, 4)]
            sa_sb = [cachep.tile([H, sz, W], BF16, tag=f"sa{b}", name=f"sa{b}")
                     for b, (_, sz) in enumerate(conv_blocks)]
            # sa duplicated along a trailing len-2 axis: the apply
            # tensor_tensor then reads packed bf16 pairs (2x DVE mode).
            sa_dup = [cachep.tile([P, sz // 2, W, 2], BF16, tag=f"sad{b}",
                                  name=f"sad{b}")
                      for b, (_, sz) in enumerate(conv_blocks)]

            # ---------------- pass 1: stream + cast + stats ----------------
            # (HWDGE f32 loads + ACT casts; a casting SWDGE DMA was tried
            # and runs ~30% below line rate — the ACT cast hides fully)
            psum_stats = ps_stats.tile([1, 8, C], F32, tag="stats")
            n_wg = W // 8
            mm_i = 0
            n_mm = PAIRS * n_wg
            for j in range(PAIRS):
                for wh in range(2):
                    st = stagep.tile([P, W2, C], F32, tag="stage")
                    nc.sync.dma_start(
                        out=st.rearrange("p w c -> p (w c)"),
                        in_=xs[2 * j:2 * j + 2, :, wh * W2:(wh + 1) * W2, :]
                        .rearrange("d h w c -> (d h) (w c)"))
                    if wh == 0:
                        nc.scalar.copy(
                            out=cache[j][:, 0:W2, :], in_=st)
                    else:
                        nc.vector.tensor_copy(
                            out=cache[j][:, W2:, :], in_=st)
                    # channel max: running elementwise max over half tiles
                    nc.vector.tensor_tensor(
                        out=acc_max[:].rearrange("p w c -> p (w c)"),
                        in0=acc_max[:].rearrange("p w c -> p (w c)"),
                        in1=cache[j][:, wh * W2:(wh + 1) * W2, :]
                        .rearrange("p w c -> p (w c)"),
                        op=OP.max)
                for g in range(n_wg):
                    nc.tensor.matmul(
                        out=psum_stats,
                        lhsT=ones[:, :],
                        rhs=cache[j][:, g * 8:(g + 1) * 8, :],
                        start=(mm_i == 0), stop=(mm_i == n_mm - 1))
                    mm_i += 1

            # finalize stats (mean scale applied here, off the critical path)
            sumc = workp.tile([1, C], F32, tag="sumc", bufs=1)
            nc.vector.tensor_reduce(
                out=sumc, in_=psum_stats[:, :, :].transpose([0, 2, 1]),
                axis=AX.X, op=OP.add)
            nc.scalar.mul(out=sumc, in_=sumc,
                          mul=1.0 / float(cfg.D_TOT * H * W))
            # fold acc_max [P, W2, C] over W2 by in-place halving (2x TTs)
            wfold = W2
            while wfold > 1:
                wfold //= 2
                nc.vector.tensor_tensor(
                    out=acc_max[:, 0:wfold, :].rearrange("p w c -> p (w c)"),
                    in0=acc_max[:, 0:wfold, :].rearrange("p w c -> p (w c)"),
                    in1=acc_max[:, wfold:2 * wfold, :]
                    .rearrange("p w c -> p (w c)"),
                    op=OP.max)
            maxr = workp.tile([P, C], F32, tag="maxr", bufs=1)
            nc.gpsimd.partition_all_reduce(
                out_ap=maxr, in_ap=acc_max[:, 0, :], channels=P,
                reduce_op=bass_isa.ReduceOp.max)

            snd = dram.tile([2, C], F32, tag="snd")
            rcv = dram.tile([2, 2, C], F32, tag="rcv")
            nc.sync.dma_start(out=snd[0:1, :], in_=sumc)
            nc.sync.dma_start(out=snd[1:2, :], in_=maxr[0:1, :])
            if cfg.use_collectives:
                nc.gpsimd.collective_compute(
                    "AllGather", OP.bypass, replica_groups=groups,
                    ins=[snd.opt()], outs=[rcv.opt()])
            else:
                nc.gpsimd.dma_start(out=rcv[0], in_=snd)
                nc.gpsimd.dma_start(out=rcv[1], in_=snd)

            # ---------------- MLP -> ca (transpose-free) ----------------
            if cfg.stop_after == "pass1":
                return nc
            # land stats transposed: quadT[c, k, r] = rcv[r, k, c]
            quadT = workp.tile([C, 2, 2], F32, tag="quadT", bufs=1)
            for r in range(2):
                nc.sync.dma_start(out=quadT[:, :, r],
                                  in_=rcv[r].rearrange("k c -> c k"))
            pooled2 = workp.tile([C, 2], F32, tag="pooled2", bufs=1)
            nc.vector.tensor_add(out=pooled2[:, 0:1], in0=quadT[:, 0, 0:1],
                                 in1=quadT[:, 0, 1:2])
            nc.vector.tensor_tensor(out=pooled2[:, 1:2], in0=quadT[:, 1, 0:1],
                                    in1=quadT[:, 1, 1:2], op=OP.max)

            psum_h = ps_sm.tile([HID, 2], F32, tag="small")
            nc.tensor.matmul(out=psum_h, lhsT=w1_sb, rhs=pooled2,
                             start=True, stop=True)
            h2 = workp.tile([HID, 2], F32, tag="h2", bufs=1)
            nc.scalar.activation(out=h2, in_=psum_h, func=ACT.Relu,
                                 bias=b1t_sb)
            # psum_ca = h2.T @ w2 + 1x2.T @ b2 (bias folded in as a matmul)
            psum_ca = ps_sm.tile([2, C], F32, tag="small")
            nc.tensor.matmul(out=psum_ca, lhsT=h2, rhs=w2_sb,
                             start=True, stop=False)
            nc.tensor.matmul(out=psum_ca, lhsT=ones12, rhs=b2b[0:1, :],
                             start=False, stop=True)
            ca2 = workp.tile([2, C], BF16, tag="ca2", bufs=1)
            nc.scalar.activation(out=ca2, in_=psum_ca, func=ACT.Sigmoid)
            car = workp.tile([2, C], BF16, tag="car", bufs=1)
            nc.gpsimd.partition_all_reduce(
                out_ap=car, in_ap=ca2, channels=2,
                reduce_op=bass_isa.ReduceOp.add)
            ca_bf = consts.tile([P, C], BF16, tag="ca_bf")
            nc.gpsimd.partition_broadcast(out_ap=ca_bf, in_ap=car[0:1, :])

            # ---------------- phase 2: pooled + conv + apply ----------------
            if cfg.stop_after == "mlp":
                return nc

            def emit_pair_phase2a(j):
                """refine in-place; SUM tree to [P,W,4] + MAX tree to [P,W]
                (DVE, all 2x); perm matmuls (PE) fold the final 4-way sum
                via PSUM accumulation; psum->pooled copies (ACT)."""
                s_e, s_o = HALO + 2 * j, HALO + 2 * j + 1
                nc.vector.tensor_mul(
                    out=cache[j], in0=cache[j],
                    in1=_bc(ca_bf[:, :], [P, W, C], 1))
                # SUM tree: halve C 64 -> 4 (stays 2x throughout)
                t1s = treep.tile([P, W, C // 2], BF16, tag="t1add",
                                 name=f"t1add_{j}")
                with nc.allow_low_precision(reason="bf16 pooled stats"):
                    nc.vector.tensor_tensor(
                        out=t1s, in0=cache[j][:, :, 0:C // 2],
                        in1=cache[j][:, :, C // 2:], op=OP.add)
                    cf = C // 2
                    while cf > 4:
                        cf //= 2
                        nc.vector.tensor_tensor(
                            out=t1s[:, :, 0:cf], in0=t1s[:, :, 0:cf],
                            in1=t1s[:, :, cf:2 * cf], op=OP.add)
                # MAX tree: halve C 64 -> 1 (stays 2x throughout)
                t1m = treep.tile([P, W, C // 2], BF16, tag="t1max",
                                 name=f"t1max_{j}")
                rpm = workp.tile([P, W], BF16, tag="rpmax",
                                 name=f"rpmax_{j}")
                nc.vector.tensor_tensor(
                    out=t1m, in0=cache[j][:, :, 0:C // 2],
                    in1=cache[j][:, :, C // 2:], op=OP.max)
                cf = C // 2
                while cf > 4:
                    cf //= 2
                    nc.vector.tensor_tensor(
                        out=t1m[:, :, 0:cf], in0=t1m[:, :, 0:cf],
                        in1=t1m[:, :, cf:2 * cf], op=OP.max)
                nc.vector.tensor_reduce(
                    out=rpm, in_=t1m[:, :, 0:4], axis=AX.X, op=OP.max)
                # perm matmuls; the 4 leftover sum groups accumulate in PSUM
                for qa, qb, slot, nm in ((qa_e, qb_e, s_e, "pe"),
                                         (qa_o, qb_o, s_o, "po")):
                    pp = ps_perm.tile([P, W], F32, tag="perm",
                                      name=f"{nm}{j}")
                    for k in range(4):
                        nc.tensor.matmul(out=pp, lhsT=qa,
                                         rhs=t1s[:, :, k],
                                         start=(k == 0), stop=False)
                    nc.tensor.matmul(out=pp, lhsT=qb, rhs=rpm,
                                     start=False, stop=True)
                    nc.scalar.copy(out=pooled[:, slot, HALO:HALO + W], in_=pp)

            # edge pairs first: they feed the pooled-halo exchange
            pair_order = [0, 1, PAIRS - 2, PAIRS - 1] + list(range(2, PAIRS - 2))
            emitted = 0
            while emitted < 4:
                emit_pair_phase2a(pair_order[emitted])
                emitted += 1

            # ---- pooled-map halo exchange (pair-wise) ----
            snd_h = dram.tile([P, 6 * WP], BF16, tag="snd_h")
            rcv_h = dram.tile([2, P, 6 * WP], BF16, tag="rcv_h")
            nc.sync.dma_start(
                out=snd_h[:, 0:3 * WP],
                in_=pooled[:, HALO:2 * HALO, :].rearrange("p s w -> p (s w)"))
            nc.sync.dma_start(
                out=snd_h[:, 3 * WP:],
                in_=pooled[:, S - 2 * HALO:S - HALO, :]
                .rearrange("p s w -> p (s w)"))
            if cfg.use_collectives:
                nc.gpsimd.collective_compute(
                    "AllGather", OP.bypass, replica_groups=groups,
                    ins=[snd_h.opt()], outs=[rcv_h.opt()])
            else:
                nc.gpsimd.dma_start(out=rcv_h[0], in_=snd_h)
                nc.gpsimd.dma_start(out=rcv_h[1], in_=snd_h)
            par = nc.sync.partition_id() & 1
            # half 0: my top halo slots <- neighbor's first 3 own planes
            nc.sync.dma_start(
                out=pooled[:, S - HALO:S, :].rearrange("p s w -> p (s w)"),
                in_=rcv_h[1, :, 0:3 * WP], cond=1 - par)
            # half 1: my low halo slots <- neighbor's last 3 own planes
            nc.sync.dma_start(
                out=pooled[:, 0:HALO, :].rearrange("p s w -> p (s w)"),
                in_=rcv_h[0, :, 3 * WP:], cond=par)

            def emit_conv_blk(blk, start, sz):
                pcv = ps_cv.tile([H, sz, W], F32, tag="cv", name=f"cv{blk}")
                k = 0
                for kd in range(KS):
                    for kw in range(KS):
                        nc.tensor.matmul(
                            out=pcv,
                            lhsT=sband_sb[:, kd * KS + kw, :],
                            rhs=pooled[:, start + kd: start + kd + sz,
                                       kw:kw + W],
                            start=(k == 0), stop=(k == NT - 1),
                            skip_group_check=True)
                        k += 1
                nc.scalar.activation(out=sa_sb[blk], in_=pcv, func=ACT.Sigmoid)
                sa_ev = sa_sb[blk].rearrange("h (a b) w -> h a b w", b=2)
                psp = ps_psp.tile([P, sz // 2, W], F32, tag="psp",
                                  name=f"psp{blk}")
                nc.tensor.matmul(out=psp, lhsT=qa_e[0:H, :],
                                 rhs=sa_ev[:, :, 0, :], start=True, stop=False)
                nc.tensor.matmul(out=psp, lhsT=qb_e[0:H, :],
                                 rhs=sa_ev[:, :, 1, :], start=False, stop=True)
                # duplicate along a trailing len-2 axis while copying out
                nc.scalar.copy(
                    out=sa_dup[blk],
                    in_=_bc(psp, [P, sz // 2, W, 2], 3))

            need_emit = [8, 12, 16, 16, 16]  # pairs done before conv blk
            for blk, (start, sz) in enumerate(conv_blocks):
                while emitted < need_emit[blk]:
                    emit_pair_phase2a(pair_order[emitted])
                    emitted += 1
                emit_conv_blk(blk, start, sz)
                for j in range(start // 2, start // 2 + sz // 2):
                    dp = j - start // 2
                    cv = cache[j].rearrange("p w (a b) -> p w a b", b=2)
                    nc.vector.tensor_tensor(
                        out=cv, in0=cv,
                        in1=_bc(sa_dup[blk][:, dp], [P, W, C // 2, 2], 2),
                        op=OP.mult)
                    nc.sync.dma_start(
                        out=out_t[2 * j:2 * j + 2]
                        .rearrange("d h w c -> (d h) (w c)"),
                        in_=cache[j].rearrange("p w c -> p (w c)"))
    return nc


def make_sband(conv_w, cfg: Cfg):
    """Host-side band-matrix construction: [P, KS*KS, H] bf16.

    sband[ci*H+h', kd*KS+kw, h] = conv_w[kd, h'-h+halo, kw, ci] (avg rows
    pre-scaled by 1/C because the pooled map stores channel sums)."""
    H, C, KS, HALO = cfg.H, cfg.C, cfg.KS, cfg.HALO
    cw = np.asarray(conv_w, np.float32)[..., 0]        # [KS,KS,KS,2]
    sb = np.zeros((cfg.P, KS * KS, H), np.float32)
    h = np.arange(H)
    for kd in range(KS):
        for kw in range(KS):
            for ci in range(2):
                scale = (1.0 / C) if ci == 0 else 1.0
                for kh in range(KS):
                    hp = h + kh - HALO                  # h' = h + kh - halo
                    m = (hp >= 0) & (hp < H)
                    sb[ci * H + hp[m], kd * KS + kw, h[m]] = cw[kd, kh, kw, ci] * scale
    return sb.astype(ml_dtypes.bfloat16)


def make_core_inputs(x, w1, b1, w2, b2, sband_np, cfg: Cfg):
    """Shard the full inputs into per-core in_maps (no halo padding)."""
    C, D_LOC = cfg.C, cfg.D_LOC
    x = np.ascontiguousarray(np.asarray(x, np.float32))
    in_maps = []
    for core in range(cfg.N_CORES):
        b, half = core // 2, core % 2
        d0 = half * D_LOC
        in_maps.append({
            "xs": x[b, d0:d0 + D_LOC],
            "w1": np.asarray(w1, np.float32).reshape(C, cfg.HID),
            "b1t": np.asarray(b1, np.float32).reshape(cfg.HID, 1),
            "w2": np.asarray(w2, np.float32).reshape(cfg.HID, C),
            "b2": np.asarray(b2, np.float32).reshape(1, C),
            "sband": sband_np,
        })
    return in_maps


_COMPILED = {}


def get_compiled(cfg: Cfg = FULL):
    if cfg not in _COMPILED:
        nc = bacc.Bacc("TRN2", target_bir_lowering=False, debug=False,
                       num_devices=cfg.N_CORES)
        build_cbam(nc, cfg)
        nc.compile()
        _COMPILED[cfg] = nc
    return _COMPILED[cfg]


def kernel(x, w1, b1, w2, b2, conv_w):
    from concourse.bass_utils import run_bass_kernel_spmd

    cfg = FULL
    nc = get_compiled(cfg)
    sband_np = make_sband(conv_w, cfg)
    in_maps = make_core_inputs(x, w1, b1, w2, b2, sband_np, cfg)
    res = run_bass_kernel_spmd(nc, in_maps, list(range(cfg.N_CORES)))
    B, D = 4, 64
    out = np.empty((B, D, cfg.H, cfg.W, cfg.C), np.float32)
    for core in range(cfg.N_CORES):
        b, half = core // 2, core % 2
        d0 = half * cfg.D_LOC
        out[b, d0:d0 + cfg.D_LOC] = np.asarray(
            res.results[core]["out"], dtype=np.float32)
    return out
